# revision 31
# baseline (speedup 1.0000x reference)
"""Sliding-window attention (window=256) on 8 TRN2 NeuronCores, bf16 pipeline.

v2: V-stationary PV + transposed output + big-descriptor DMA layouts.

Layout/algorithm notes
----------------------
Shapes: q,k,v [4,16,4096,64]; B*H=64 (b,h) pairs sharded 8 per core
(fully local along sequence, no communication).  The host pre-casts to
bf16 and pre-transposes Q/K to [D, S]; V is host-packed to the k-major
chunk layout [128, 32, 65] with a ones column baked in (so every DMA is
one contiguous descriptor per partition).

Per (b,h) and per 512-query block t (8 per head), 6 key chunks of 128
(global chunk g = 4t-2+c, c=0..5; g<0 skipped):
  S^T chunk = matmul(lhsT=K^T[:,128g:+128] [64,128],
                     rhs=Q^T[:, 512t+qw_c]  [64,|qw_c|])
  written into ONE 3-bank PSUM tile [128,1536] in chunk order
  [c0|c2|c1|c4|c3|c5] (pairs share a 512-col bank).  A SINGLE wide ACT
  exp (scale=D^-1/2, PSUM->SBUF bf16) and a SINGLE wide DVE band-mask
  multiply produce P^T [128,1536].

  PV is V-STATIONARY (the key change vs v1): per 128-query tile j,
  members c in {j, j+1, j+2} accumulate
    O^T[65, 128q] += matmul(lhsT=[V|1] chunk [128k, 65], rhs=P^T slice)
  so the PE loads only 65-column weights (vs 128-column P^T slices) and
  P^T streams through the array at full matmul rate instead of at
  weight-load rate.  Row 64 of O^T holds the softmax denominator.
  Epilogue per block: one DVE copy [65,512] PSUM->SBUF bf16 into a
  per-(b,h) O^T slab [65, S]; one contiguous store DMA per (b,h).

The HOST performs the final (numerator / denominator) and [65,S]->[S,64]
transpose when gathering (softmax normalize division: ~0.05% of the
kernel FLOPs; all matmuls, exp, masking and reductions run on-chip).

Emission is software-pipelined (QK/exp/mask of block t ahead of PV/copy
of t-1) across all (b,h) so the PE never drains at a head boundary."""

import numpy as np
import ml_dtypes

import concourse.bass as bass
import concourse.mybir as mybir
from concourse import bacc
from concourse.tile import TileContext
from concourse import bass_utils

dt = mybir.dt

B, H, S, D = 4, 16, 4096, 64
W = 256                      # sliding window
N_CORES = 8
BH = (B * H) // N_CORES      # (b,h) pairs per core = 8
QT = 512                     # queries per block
NB = S // QT                 # blocks per (b,h) = 8
NG = S // 128                # 128-key chunks per (b,h) = 32
SCALE = float(D) ** -0.5
E = D + 1                    # V columns + ones column

# chunk order within the S^T / P^T row of banks: pairs share a 512-col bank
ORDER = [0, 2, 1, 4, 3, 5]
# per-chunk query windows (relative to block start), c = 0..5
W0 = [max(0, 128 * (c - 2)) for c in range(6)]
W1 = [min(QT, 128 * (c - 2) + 384) for c in range(6)]
BASE = {}
_off = 0
for _c in ORDER:
    BASE[_c] = _off
    _off += W1[_c] - W0[_c]
PT_W = _off
assert PT_W == 1536


def _mega_mask_np():
    """[128, 1536] multiplicative band mask matching the pt layout.
    Entry (kl, BASE[c] + q - W0[c]) is 1 iff 0 <= q + 128*(2-c) - kl <= 256
    for q in [W0[c], W1[c])."""
    m = np.zeros((128, PT_W), dtype=np.float32)
    kl = np.arange(128)[:, None]
    for c in range(6):
        q = np.arange(W0[c], W1[c])[None, :]
        rel = q + 128 * (2 - c) - kl
        m[:, BASE[c]:BASE[c] + W1[c] - W0[c]] = (
            (rel >= 0) & (rel <= W)).astype(np.float32)
    return m


def build_core_kernel(n_bh=BH):
    nc = bacc.Bacc("TRN2", target_bir_lowering=False)
    # q/k arrive HOST-TRANSPOSED: per (b,h) a [D, S] slab
    qd = nc.dram_tensor("qT", [n_bh * D, S], dt.bfloat16, kind="ExternalInput")
    kd = nc.dram_tensor("kT", [n_bh * D, S], dt.bfloat16, kind="ExternalInput")
    # V host-packed: [bh, 128, NG*E]; (p, g, e<D) = v[128g+p, e]; e==D -> 1.0
    vd = nc.dram_tensor("v1", [n_bh * 128, NG * E], dt.bfloat16,
                        kind="ExternalInput")
    md = nc.dram_tensor("band_mask", [128, PT_W], dt.bfloat16,
                        kind="ExternalInput")
    # output TRANSPOSED per (b,h): [65, S]; rows 0..63 numerator^T, row 64
    # the softmax denominator (host divides + transposes)
    od = nc.dram_tensor("oT", [n_bh * E, S], dt.bfloat16,
                        kind="ExternalOutput")

    with TileContext(nc) as tc:
        with (
            tc.tile_pool(name="const", bufs=1) as constp,
            tc.tile_pool(name="bigio", bufs=3) as bigio,
            tc.tile_pool(name="work", bufs=3) as work,
            tc.tile_pool(name="psst", bufs=2, space="PSUM") as psst,
            tc.tile_pool(name="psoq", bufs=2, space="PSUM") as psoq,
        ):
            mega = constp.tile([128, PT_W], dt.bfloat16)
            nc.gpsimd.dma_start(mega[:], md[:])

            tiles = {}   # bh -> (qt, kt, vt3, otsb, pts)

            def emit_loads(bh):
                qt = bigio.tile([64, S], dt.bfloat16, tag="qt", name="qt")
                kt = bigio.tile([64, S], dt.bfloat16, tag="kt", name="kt")
                if bh == 0:
                    # urgent first block (512 q cols / first 4 key chunks) so
                    # QK of block 0 starts as early as possible
                    for c0, c1 in ((0, 512), (512, 1024), (1024, 2048),
                                   (2048, S)):
                        nc.sync.dma_start(qt[:, c0:c1],
                                          qd[bh * D:(bh + 1) * D, c0:c1])
                        nc.sync.dma_start(kt[:, c0:c1],
                                          kd[bh * D:(bh + 1) * D, c0:c1])
                else:
                    nc.sync.dma_start(qt[:], qd[bh * D:(bh + 1) * D, :])
                    nc.sync.dma_start(kt[:], kd[bh * D:(bh + 1) * D, :])
                vt = bigio.tile([128, NG * E], dt.bfloat16, tag="vt",
                                name="vt")
                nc.sync.dma_start(vt[:], vd[bh * 128:(bh + 1) * 128, :])
                vt3 = vt[:].rearrange("p (g e) -> p g e", e=E)
                # O^T slab [65, S] bf16 per (b,h)
                otsb = bigio.tile([E, S], dt.bfloat16, tag="otsb",
                                  name="otsb")
                tiles[bh] = (qt, kt, vt3, otsb, {})

            def emit_qk(bh, t):
                """QK chunk matmuls into one 3-bank S^T tile + single
                wide exp + single wide band-mask multiply."""
                qt, kt, vt3, otsb, pts = tiles[bh]
                st = psst.tile([128, PT_W], dt.float32, tag="st", name="st")
                for c in range(6):
                    g = 4 * t - 2 + c
                    if g < 0:
                        continue
                    w = W1[c] - W0[c]
                    nc.tensor.matmul(
                        st[:, BASE[c]:BASE[c] + w],
                        kt[:, 128 * g:128 * (g + 1)],
                        qt[:, QT * t + W0[c]:QT * t + W1[c]],
                        start=True, stop=True)
                pt = work.tile([128, PT_W], dt.bfloat16, tag="pt", name="pt")
                pts[t] = pt
                nc.scalar.activation(
                    pt[:], st[:], mybir.ActivationFunctionType.Exp,
                    scale=SCALE)
                nc.vector.tensor_tensor(
                    pt[:], pt[:], mega[:], op=mybir.AluOpType.mult)

            ots = {}     # i -> (ot psum tile, otsb, t) pending evacuation

            def emit_pv(bh, t):
                """V-stationary PV: O^T[65, 512] per block; j-major
                accumulation (each j group start->stop before the next
                starts, so whole-bank has_written clears are safe)."""
                qt, kt, vt3, otsb, pts = tiles[bh]
                pt = pts.pop(t)
                ot = psoq.tile([E, QT], dt.float32, tag="ot", name="ot")
                for j in range(4):
                    members = [c for c in (j, j + 1, j + 2)
                               if 4 * t - 2 + c >= 0]
                    for mi, c in enumerate(members):
                        g = 4 * t - 2 + c
                        po = BASE[c] + 128 * j - W0[c]
                        nc.tensor.matmul(
                            ot[:, 128 * j:128 * (j + 1)],
                            vt3[:, g, :],
                            pt[:, po:po + 128],
                            start=(mi == 0), stop=(mi == len(members) - 1))
                ots[bh * NB + t] = (ot, otsb, t)

            def emit_cast(i):
                """Deferred PSUM -> O^T slab evacuation (bf16).  Emitted one
                block later than the PV so the DVE queue never waits on the
                PE's PV matmuls (keeps mask(i) off the PE critical path)."""
                ot, otsb, t = ots.pop(i)
                nc.vector.tensor_copy(otsb[:, QT * t:QT * (t + 1)], ot[:])

            def emit_store(bh, t0=0, t1=NB, pop=True):
                ent = tiles.pop(bh) if pop else tiles[bh]
                otsb = ent[3]
                nc.sync.dma_start(
                    od[bh * E:(bh + 1) * E, QT * t0:QT * t1],
                    otsb[:, QT * t0:QT * t1])

            # one flat software pipeline across ALL (b,h): QK/exp/mask of
            # global block i, PV of block i-1, PSUM evacuation of block i-2,
            # crossing (b,h) boundaries so the PE never drains at a head
            # boundary
            NTOT = n_bh * NB

            def after_cast(i2):
                bh2, t2 = divmod(i2, NB)
                if bh2 == n_bh - 1:
                    # drain the last head eagerly: store each tail block as
                    # soon as its cast lands so the final DMA is tiny
                    if t2 == NB - 4:
                        emit_store(bh2, 0, NB - 3, pop=False)
                    elif t2 >= NB - 3:
                        emit_store(bh2, t2, t2 + 1, pop=(t2 == NB - 1))
                elif t2 == NB - 1:
                    emit_store(bh2)

            for i in range(NTOT):
                bh, t = divmod(i, NB)
                if t == 0:
                    emit_loads(bh)
                emit_qk(bh, t)
                if i >= 1:
                    emit_pv(*divmod(i - 1, NB))
                if i >= 2:
                    emit_cast(i - 2)
                    after_cast(i - 2)
            emit_pv(n_bh - 1, NB - 1)
            emit_cast(NTOT - 2)
            after_cast(NTOT - 2)
            emit_cast(NTOT - 1)
            after_cast(NTOT - 1)

    nc.finalize()
    return nc


_NC_CACHE = []


def _get_nc():
    if not _NC_CACHE:
        _NC_CACHE.append(build_core_kernel())
    return _NC_CACHE[0]


def make_in_maps(q, k, v):
    bf16 = ml_dtypes.bfloat16
    qr = np.asarray(q, dtype=np.float32).reshape(B * H, S, D)
    kr = np.asarray(k, dtype=np.float32).reshape(B * H, S, D)
    vr = np.asarray(v, dtype=np.float32).reshape(B * H, S, D)
    # host-side transpose to [bh, D, S] for direct Q^T/K^T loads
    qT = np.ascontiguousarray(qr.transpose(0, 2, 1)).astype(bf16)
    kT = np.ascontiguousarray(kr.transpose(0, 2, 1)).astype(bf16)
    # host-packed V: [bh, p=128, g=NG, e=E] with ones column at e=D
    v1 = np.ones((B * H, 128, NG, E), dtype=np.float32)
    v1[:, :, :, :D] = vr.reshape(B * H, NG, 128, D).transpose(0, 2, 1, 3)
    v1 = v1.astype(bf16)
    band = np.ascontiguousarray(_mega_mask_np().astype(bf16))

    in_maps = []
    for i in range(N_CORES):
        in_maps.append({
            "qT": np.ascontiguousarray(
                qT[BH * i:BH * (i + 1)].reshape(BH * D, S)),
            "kT": np.ascontiguousarray(
                kT[BH * i:BH * (i + 1)].reshape(BH * D, S)),
            "v1": np.ascontiguousarray(
                v1[BH * i:BH * (i + 1)].reshape(BH * 128, NG * E)),
            "band_mask": band,
        })
    return in_maps


def gather_out(res):
    out = np.empty((B * H, S, D), dtype=np.float32)
    for i in range(N_CORES):
        oT = np.asarray(res.results[i]["oT"],
                        dtype=np.float32).reshape(BH, E, S)
        # host epilogue: softmax normalize (divide by denominator row)
        # and [E, S] -> [S, D] transpose
        out[BH * i:BH * (i + 1)] = (
            oT[:, :D, :] / oT[:, D:D + 1, :]).transpose(0, 2, 1)
    return out.reshape(B, H, S, D)


def kernel(q, k, v):
    nc = _get_nc()
    in_maps = make_in_maps(q, k, v)
    res = bass_utils.run_bass_kernel_spmd(nc, in_maps, core_ids=list(range(N_CORES)))
    return gather_out(res)


# revision 32
# speedup vs baseline: 1.0154x; 1.0154x over previous
"""Sliding-window attention (window=256) on 8 TRN2 NeuronCores, bf16 pipeline.

v2: V-stationary PV + transposed output + big-descriptor DMA layouts.

Layout/algorithm notes
----------------------
Shapes: q,k,v [4,16,4096,64]; B*H=64 (b,h) pairs sharded 8 per core
(fully local along sequence, no communication).  The host pre-casts to
bf16 and pre-transposes Q/K to [D, S]; V is host-packed to the k-major
chunk layout [128, 32, 65] with a ones column baked in (so every DMA is
one contiguous descriptor per partition).

Per (b,h) and per 512-query block t (8 per head), 6 key chunks of 128
(global chunk g = 4t-2+c, c=0..5; g<0 skipped):
  S^T chunk = matmul(lhsT=K^T[:,128g:+128] [64,128],
                     rhs=Q^T[:, 512t+qw_c]  [64,|qw_c|])
  written into ONE 3-bank PSUM tile [128,1536] in chunk order
  [c0|c2|c1|c4|c3|c5] (pairs share a 512-col bank).  A SINGLE wide ACT
  exp (scale=D^-1/2, PSUM->SBUF bf16) and a SINGLE wide DVE band-mask
  multiply produce P^T [128,1536].

  PV is V-STATIONARY (the key change vs v1): per 128-query tile j,
  members c in {j, j+1, j+2} accumulate
    O^T[65, 128q] += matmul(lhsT=[V|1] chunk [128k, 65], rhs=P^T slice)
  so the PE loads only 65-column weights (vs 128-column P^T slices) and
  P^T streams through the array at full matmul rate instead of at
  weight-load rate.  Row 64 of O^T holds the softmax denominator.
  Epilogue per block: one DVE copy [65,512] PSUM->SBUF bf16 into a
  per-(b,h) O^T slab [65, S]; one contiguous store DMA per (b,h).

The HOST performs the final (numerator / denominator) and [65,S]->[S,64]
transpose when gathering (softmax normalize division: ~0.05% of the
kernel FLOPs; all matmuls, exp, masking and reductions run on-chip).

Emission is software-pipelined (QK/exp/mask of block t ahead of PV/copy
of t-1) across all (b,h) so the PE never drains at a head boundary."""

import numpy as np
import ml_dtypes

import concourse.bass as bass
import concourse.mybir as mybir
from concourse import bacc
from concourse.tile import TileContext
from concourse import bass_utils

dt = mybir.dt

B, H, S, D = 4, 16, 4096, 64
W = 256                      # sliding window
N_CORES = 8
BH = (B * H) // N_CORES      # (b,h) pairs per core = 8
QT = 512                     # queries per block
NB = S // QT                 # blocks per (b,h) = 8
NG = S // 128                # 128-key chunks per (b,h) = 32
SCALE = float(D) ** -0.5
E = D + 1                    # V columns + ones column

# chunk order within the S^T / P^T row of banks: pairs share a 512-col bank
ORDER = [0, 2, 1, 4, 3, 5]
# per-chunk query windows (relative to block start), c = 0..5
W0 = [max(0, 128 * (c - 2)) for c in range(6)]
W1 = [min(QT, 128 * (c - 2) + 384) for c in range(6)]
BASE = {}
_off = 0
for _c in ORDER:
    BASE[_c] = _off
    _off += W1[_c] - W0[_c]
PT_W = _off
assert PT_W == 1536


def _mega_mask_np():
    """[128, 1536] multiplicative band mask matching the pt layout.
    Entry (kl, BASE[c] + q - W0[c]) is 1 iff 0 <= q + 128*(2-c) - kl <= 256
    for q in [W0[c], W1[c])."""
    m = np.zeros((128, PT_W), dtype=np.float32)
    kl = np.arange(128)[:, None]
    for c in range(6):
        q = np.arange(W0[c], W1[c])[None, :]
        rel = q + 128 * (2 - c) - kl
        m[:, BASE[c]:BASE[c] + W1[c] - W0[c]] = (
            (rel >= 0) & (rel <= W)).astype(np.float32)
    return m


def build_core_kernel(n_bh=BH):
    nc = bacc.Bacc("TRN2", target_bir_lowering=False)
    # q/k arrive HOST-TRANSPOSED: per (b,h) a [D, S] slab
    qd = nc.dram_tensor("qT", [n_bh * D, S], dt.bfloat16, kind="ExternalInput")
    kd = nc.dram_tensor("kT", [n_bh * D, S], dt.bfloat16, kind="ExternalInput")
    # V host-packed: [bh, 128, NG*E]; (p, g, e<D) = v[128g+p, e]; e==D -> 1.0
    vd = nc.dram_tensor("v1", [n_bh * 128, NG * E], dt.bfloat16,
                        kind="ExternalInput")
    md = nc.dram_tensor("band_mask", [128, PT_W], dt.bfloat16,
                        kind="ExternalInput")
    # output TRANSPOSED per (b,h): [65, S]; rows 0..63 numerator^T, row 64
    # the softmax denominator (host divides + transposes)
    od = nc.dram_tensor("oT", [n_bh * E, S], dt.bfloat16,
                        kind="ExternalOutput")

    with TileContext(nc) as tc:
        with (
            tc.tile_pool(name="const", bufs=1) as constp,
            tc.tile_pool(name="bigio", bufs=3) as bigio,
            tc.tile_pool(name="work", bufs=3) as work,
            tc.tile_pool(name="psst", bufs=2, space="PSUM") as psst,
            tc.tile_pool(name="psoq", bufs=2, space="PSUM") as psoq,
        ):
            mega = constp.tile([128, PT_W], dt.bfloat16)
            nc.gpsimd.dma_start(mega[:], md[:])

            tiles = {}   # bh -> (qt, kt, vt3, otsb, pts)

            def emit_loads(bh):
                qt = bigio.tile([64, S], dt.bfloat16, tag="qt", name="qt")
                kt = bigio.tile([64, S], dt.bfloat16, tag="kt", name="kt")
                if bh == 0:
                    # urgent first block (512 q cols / first 4 key chunks) so
                    # QK of block 0 starts as early as possible
                    for c0, c1 in ((0, 512), (512, 1024), (1024, S)):
                        nc.sync.dma_start(qt[:, c0:c1],
                                          qd[bh * D:(bh + 1) * D, c0:c1])
                        nc.sync.dma_start(kt[:, c0:c1],
                                          kd[bh * D:(bh + 1) * D, c0:c1])
                else:
                    nc.sync.dma_start(qt[:], qd[bh * D:(bh + 1) * D, :])
                    nc.sync.dma_start(kt[:], kd[bh * D:(bh + 1) * D, :])
                vt = bigio.tile([128, NG * E], dt.bfloat16, tag="vt",
                                name="vt")
                nc.sync.dma_start(vt[:], vd[bh * 128:(bh + 1) * 128, :])
                vt3 = vt[:].rearrange("p (g e) -> p g e", e=E)
                # O^T slab [65, S] bf16 per (b,h)
                otsb = bigio.tile([E, S], dt.bfloat16, tag="otsb",
                                  name="otsb")
                tiles[bh] = (qt, kt, vt3, otsb, {})

            def emit_qk(bh, t):
                """QK chunk matmuls into one 3-bank S^T tile + single
                wide exp + single wide band-mask multiply."""
                qt, kt, vt3, otsb, pts = tiles[bh]
                st = psst.tile([128, PT_W], dt.float32, tag="st", name="st")
                for c in range(6):
                    g = 4 * t - 2 + c
                    if g < 0:
                        continue
                    w = W1[c] - W0[c]
                    nc.tensor.matmul(
                        st[:, BASE[c]:BASE[c] + w],
                        kt[:, 128 * g:128 * (g + 1)],
                        qt[:, QT * t + W0[c]:QT * t + W1[c]],
                        start=True, stop=True)
                pt = work.tile([128, PT_W], dt.bfloat16, tag="pt", name="pt")
                pts[t] = pt
                nc.scalar.activation(
                    pt[:], st[:], mybir.ActivationFunctionType.Exp,
                    scale=SCALE)
                nc.vector.tensor_tensor(
                    pt[:], pt[:], mega[:], op=mybir.AluOpType.mult)

            ots = {}     # i -> (ot psum tile, otsb, t) pending evacuation

            def emit_pv(bh, t):
                """V-stationary PV: O^T[65, 512] per block; j-major
                accumulation (each j group start->stop before the next
                starts, so whole-bank has_written clears are safe)."""
                qt, kt, vt3, otsb, pts = tiles[bh]
                pt = pts.pop(t)
                ot = psoq.tile([E, QT], dt.float32, tag="ot", name="ot")
                for j in range(4):
                    members = [c for c in (j, j + 1, j + 2)
                               if 4 * t - 2 + c >= 0]
                    for mi, c in enumerate(members):
                        g = 4 * t - 2 + c
                        po = BASE[c] + 128 * j - W0[c]
                        nc.tensor.matmul(
                            ot[:, 128 * j:128 * (j + 1)],
                            vt3[:, g, :],
                            pt[:, po:po + 128],
                            start=(mi == 0), stop=(mi == len(members) - 1))
                ots[bh * NB + t] = (ot, otsb, t)

            def emit_cast(i):
                """Deferred PSUM -> O^T slab evacuation (bf16).  Emitted one
                block later than the PV so the DVE queue never waits on the
                PE's PV matmuls (keeps mask(i) off the PE critical path)."""
                ot, otsb, t = ots.pop(i)
                nc.vector.tensor_copy(otsb[:, QT * t:QT * (t + 1)], ot[:])

            def emit_store(bh, t0=0, t1=NB, pop=True):
                ent = tiles.pop(bh) if pop else tiles[bh]
                otsb = ent[3]
                nc.sync.dma_start(
                    od[bh * E:(bh + 1) * E, QT * t0:QT * t1],
                    otsb[:, QT * t0:QT * t1])

            # one flat software pipeline across ALL (b,h): QK/exp/mask of
            # global block i, PV of block i-1, PSUM evacuation of block i-2,
            # crossing (b,h) boundaries so the PE never drains at a head
            # boundary
            NTOT = n_bh * NB

            def after_cast(i2):
                bh2, t2 = divmod(i2, NB)
                if bh2 == n_bh - 1:
                    # drain the last head eagerly: store each tail block as
                    # soon as its cast lands so the final DMA is tiny
                    if t2 == NB - 4:
                        emit_store(bh2, 0, NB - 3, pop=False)
                    elif t2 >= NB - 3:
                        emit_store(bh2, t2, t2 + 1, pop=(t2 == NB - 1))
                elif t2 == NB - 1:
                    emit_store(bh2)

            for i in range(NTOT):
                bh, t = divmod(i, NB)
                if t == 0:
                    emit_loads(bh)
                emit_qk(bh, t)
                if i >= 1:
                    emit_pv(*divmod(i - 1, NB))
                if i >= 2:
                    emit_cast(i - 2)
                    after_cast(i - 2)
            emit_pv(n_bh - 1, NB - 1)
            emit_cast(NTOT - 2)
            after_cast(NTOT - 2)
            emit_cast(NTOT - 1)
            after_cast(NTOT - 1)

    nc.finalize()
    return nc


_NC_CACHE = []


def _get_nc():
    if not _NC_CACHE:
        _NC_CACHE.append(build_core_kernel())
    return _NC_CACHE[0]


def make_in_maps(q, k, v):
    bf16 = ml_dtypes.bfloat16
    qr = np.asarray(q, dtype=np.float32).reshape(B * H, S, D)
    kr = np.asarray(k, dtype=np.float32).reshape(B * H, S, D)
    vr = np.asarray(v, dtype=np.float32).reshape(B * H, S, D)
    # host-side transpose to [bh, D, S] for direct Q^T/K^T loads
    qT = np.ascontiguousarray(qr.transpose(0, 2, 1)).astype(bf16)
    kT = np.ascontiguousarray(kr.transpose(0, 2, 1)).astype(bf16)
    # host-packed V: [bh, p=128, g=NG, e=E] with ones column at e=D
    v1 = np.ones((B * H, 128, NG, E), dtype=np.float32)
    v1[:, :, :, :D] = vr.reshape(B * H, NG, 128, D).transpose(0, 2, 1, 3)
    v1 = v1.astype(bf16)
    band = np.ascontiguousarray(_mega_mask_np().astype(bf16))

    in_maps = []
    for i in range(N_CORES):
        in_maps.append({
            "qT": np.ascontiguousarray(
                qT[BH * i:BH * (i + 1)].reshape(BH * D, S)),
            "kT": np.ascontiguousarray(
                kT[BH * i:BH * (i + 1)].reshape(BH * D, S)),
            "v1": np.ascontiguousarray(
                v1[BH * i:BH * (i + 1)].reshape(BH * 128, NG * E)),
            "band_mask": band,
        })
    return in_maps


def gather_out(res):
    out = np.empty((B * H, S, D), dtype=np.float32)
    for i in range(N_CORES):
        oT = np.asarray(res.results[i]["oT"],
                        dtype=np.float32).reshape(BH, E, S)
        # host epilogue: softmax normalize (divide by denominator row)
        # and [E, S] -> [S, D] transpose
        out[BH * i:BH * (i + 1)] = (
            oT[:, :D, :] / oT[:, D:D + 1, :]).transpose(0, 2, 1)
    return out.reshape(B, H, S, D)


def kernel(q, k, v):
    nc = _get_nc()
    in_maps = make_in_maps(q, k, v)
    res = bass_utils.run_bass_kernel_spmd(nc, in_maps, core_ids=list(range(N_CORES)))
    return gather_out(res)
